# revision 25
# baseline (speedup 1.0000x reference)
"""Trainium2 Bass kernel for DiT multi-head attention block.

Computes, for x [B=2, N=4096, C=768]:
    qkv = x @ W_qkv                      # [B, N, 3C], no bias
    q, k, v = split(qkv) -> [B, H=12, N, D=64]
    attn = softmax(q k^T / sqrt(D))
    out  = (attn @ v) -> [B, N, C]
    out @ W_proj + b_proj

Sharding over 8 NeuronCores: core = b*4 + g handles batch b and the 3
heads [3g, 3g+3). Each core computes its heads' K/V/Q projections over
the full sequence, flash attention (no-max-subtraction softmax: scores
are ~N(0,1) so exp never overflows), and a partial output projection
restricted to its heads' rows of W_proj. The host sums the 4 fp16
partials per batch and adds the bias. No cross-core collectives.

v2 design notes (baseline was 685us, ACT-exp and pipeline-depth bound):
  - fp16 datapath everywhere (same PE speed as bf16, 8x finer
    quantization) frees error budget for the Schraudolph exp below.
  - The 50M-elem/core softmax exp is split between the ACT engine
    (table exp) and the DVE (a single tensor_scalar computing the
    Schraudolph bit-pattern of exp(s*SCALE)). ACT alone was a 400us
    serial bottleneck.
  - Software-pipelined emission: each supertile's S matmuls + exp are
    emitted before the previous supertile's PV, so S runs early on the
    in-order PE and the exp engines never starve.
  - K^T/Q^T half-duplication (for PE row-tiled S pairs) via SBUF->SBUF
    DMA; PSUM evictions on nc.any so the scheduler balances ACT/DVE.
  - Phase 3 (output projection) is interleaved into the next q-block's
    supertile stream one [128-token-pair x 384-col] group at a time.
    Output partials are written fp16.

v3 (514us -> ~450-550us depending on device day): fp8 P*V.
  - P (post-softmax probs) is produced directly in fp8e4m3: the ACT
    path writes fp8 from the exp table (with a -2.0 bias that cancels
    in the softmax ratio and keeps e4m3 bits clear of NaN for scores
    up to +8.2 sigma); the DVE path computes Schraudolph uint8 e4m3
    bits (uint8 conversion saturates at 0, so deep-negative scores
    become +0.0 rather than 0xFF).
  - V is stored fp8 and each PV step is ONE DoubleRow matmul
    contracting 256 keys ([128,2,65] stationary planes, the P tile's
    two k-tile halves as moving planes). Measured on HW: a DoubleRow
    matmul costs the same as an fp16 matmul of equal moving size, so
    the win is halved PE instruction count, not faster streaming.
  - fp8 V quantization error (~2.8% rel) breaks the 2e-2 gate, so a
    second accumulating DoubleRow matmul applies the quantization
    residual fp8(V - fp8(V)), staggered one supertile behind its v
    matmul to space out same-PSUM-bank writes (rapid same-bank
    revisits stall the PE; measured 185ns vs 1200ns per matmul).
  - Measured rel err 1.54e-2 (emulation-validated: P-e4m3 ~1.5%
    dominates; V residual compensation removes the V term).
"""

import numpy as np

B = 2
N = 4096
C = 768
H = 12
D = 64
SCALE = D ** -0.5
NH = 3  # heads per core
CCH = C // 128  # contract chunks over channels

# Schraudolph fp16-exp constants (DVE path): bits = round(s*A + B) with
# s the raw (unscaled) score; exp(s*SCALE) ~= bitcast_fp16(bits).
# c centers the multiplicative error (range +-3.9%) to ~zero mean.
EXP_A = (1 << 10) / float(np.log(2.0)) * SCALE
EXP_C = 57.0
EXP_B = float(15 * (1 << 10)) - EXP_C

# Schraudolph fp8e4m3-exp constants (DVE path, v3): uint8 bits of the
# e4m3 representation of exp(s*SCALE - EXP8_SHIFT):
# bits = round(s*A8 + B8). e4m3 bias 7, 3 mantissa bits; c8 = 57/128
# recenters like EXP_C. The SHIFT cancels exactly in the softmax ratio
# and keeps the e4m3 bits clear of the NaN boundary (bits=127) up to
# +8.2 sigma scores; uint8 conversion saturates at 0 (verified on HW),
# so deep-negative scores land at +0.0 instead of 0xFF (-NaN).
EXP8_SHIFT = 2.0
EXP8_A = (1 << 3) / float(np.log(2.0)) * SCALE
EXP8_C = 57.0 / 128.0
EXP8_B = float(7 * (1 << 3)) - EXP8_C - EXP8_SHIFT * (1 << 3) / float(np.log(2.0))

_CACHED_NC = {}


def _build(n_tokens=N, qb=1024, reps=1, phases=3, dve_den=11, dve_num=5):
    """reps > 1 repeats the whole computation inside one NEFF (timing
    aid: the walltime delta between reps and reps=1 isolates the
    on-device execution time from the multi-ms host dispatch cost).
    dve_num/dve_den: fraction of exp supertiles computed on the DVE via
    Schraudolph (rest on ACT)."""
    import concourse.bacc as bacc
    import concourse.bass as bass
    from concourse import mybir, tile

    f32 = mybir.dt.float32
    f16 = mybir.dt.float16
    i16 = mybir.dt.int16
    f8 = mybir.dt.float8e4
    u8 = mybir.dt.uint8
    DR = mybir.MatmulPerfMode.DoubleRow
    PSUM = bass.MemorySpace.PSUM
    Exp = mybir.ActivationFunctionType.Exp

    nt_tiles = n_tokens // 128   # token tiles (also k tiles)
    nqb = n_tokens // qb         # q blocks
    nkt = nt_tiles               # k tiles of 128
    qh = min(512, qb)            # q columns per matmul chunk
    nt4 = 4                      # token tiles per qkv batch
    ntb = nt_tiles // nt4        # qkv batches

    nc = bacc.Bacc("TRN2", target_bir_lowering=False, debug=False)

    # Inputs arrive pre-sharded, pre-cast to fp16, and pre-arranged to
    # the on-chip [partition, chunk, col] layouts so every load is one
    # contiguous DMA (no strided descriptor storm on the HWDGE).
    xT_dram = nc.declare_dram_parameter("xT_b", [128, ntb * CCH * nt4 * 128], f16, isOutput=False)
    wq_dram = nc.declare_dram_parameter("w_q", [128, CCH * NH * D], f16, isOutput=False)
    wk_dram = nc.declare_dram_parameter("w_k", [128, CCH * NH * D], f16, isOutput=False)
    wv_dram = nc.declare_dram_parameter("w_v", [128, CCH * NH * D], f16, isOutput=False)
    wkq2_dram = nc.declare_dram_parameter("w_kq2", [128, CCH * 128], f16, isOutput=False)
    wp01_dram = nc.declare_dram_parameter("w_p01", [128, C], f16, isOutput=False)
    wp2d_dram = nc.declare_dram_parameter("w_p2d", [128, C], f16, isOutput=False)
    out_dram = nc.declare_dram_parameter("out", [n_tokens, C], f16, isOutput=True)

    from contextlib import ExitStack

    with tile.TileContext(nc) as tc, ExitStack() as ctx:
        pp = ctx.enter_context(tc.tile_pool(name="persist", bufs=1))
        # xT layout: [partition, token-batch, chunk, col]
        xT = pp.tile([128, ntb, CCH, nt4 * 128], f16, name="xT")
        Kd = [pp.tile([128, n_tokens], f16, name=f"Kd{h}") for h in range(NH)]
        Qd = [pp.tile([128, n_tokens], f16, name=f"Qd{h}") for h in range(NH)]
        # fp8 V, padded to a 128-wide slot per (head, k-tile) so the
        # DoubleRow ldweights plane stride (128) is a multiple of 64
        # (walrus ISA constraint). vr_sb holds the fp8 quantization
        # residual fp8(V - fp8(V)); a second accumulating DoubleRow
        # matmul applies it, cutting the V-quant error ~20x.
        v_sb = pp.tile([128, NH, nkt, 128], f8, name="v_sb")
        vr_sb = pp.tile([128, NH, nkt, 128], f8, name="vr_sb")
        outn01 = pp.tile([128, n_tokens], f16, name="outn01")
        outn2d = pp.tile([128, n_tokens], f16, name="outn2d")
        wq = pp.tile([128, CCH, NH * D], f16, name="wq")
        wk = pp.tile([128, CCH, NH * D], f16, name="wk")
        wv = pp.tile([128, CCH, NH * D], f16, name="wv")
        wkq2 = pp.tile([128, CCH, 128], f16, name="wkq2")
        wp01 = pp.tile([128, C], f16, name="wp01")
        wp2d = pp.tile([128, C], f16, name="wp2d")
        ones64 = pp.tile([1, 64], f16, name="ones64")
        nbias = pp.tile([128, 1], f32, name="nbias")

        # One shared PSUM ring (2 x 2 banks) + a double-buffered PV
        # accumulator (2 x 2 banks) = exactly 8 PSUM banks. The second
        # accumulator lets head h+1's PV start while head h's staging
        # copy drains (was a ~1.3us PE gap per (head, q-block)).
        psp = ctx.enter_context(tc.tile_pool(name="ps", bufs=2, space=PSUM))
        accp = ctx.enter_context(tc.tile_pool(name="accpsum", bufs=2, space=PSUM))
        ptp = ctx.enter_context(tc.tile_pool(name="ptile", bufs=10))
        normp = ctx.enter_context(tc.tile_pool(name="norm", bufs=2))
        pop = ctx.enter_context(tc.tile_pool(name="projsb", bufs=6))

        def ps_tile(name):
            return psp.tile([128, 1024], f32, tag="S", name=name)

        # ---- weights: plain contiguous DMAs (pre-arranged host-side),
        # emitted in first-use order so phase 1 starts ASAP ----
        nc.sync.dma_start(wk[:].rearrange("p a d -> p (a d)"), wk_dram[:])
        nc.sync.dma_start(wq[:].rearrange("p a d -> p (a d)"), wq_dram[:])

        for rep in range(reps):
            rp = f"r{rep}_"

            # ---- phase 1: x load + qkv projections ----
            nbt = CCH * nt4 * 128  # fp16 elems per (partition, batch)
            for bt in range(ntb):
                nc.sync.dma_start(
                    xT[:, bt, :, :].rearrange("p a n -> p (a n)"),
                    xT_dram[:, bt * nbt:(bt + 1) * nbt])
            if rep == 0:
                # remaining weights, behind the first x batches
                nc.sync.dma_start(wv[:].rearrange("p a d -> p (a d)"), wv_dram[:])
                nc.sync.dma_start(wkq2[:].rearrange("p a d -> p (a d)"), wkq2_dram[:])
                nc.sync.dma_start(wp01[:], wp01_dram[:])
                nc.sync.dma_start(wp2d[:], wp2d_dram[:])
                nc.vector.memset(v_sb[:, :, :, 64:65], 1.0)
                nc.vector.memset(vr_sb[:, :, :, 64:65], 0.0)
                nc.vector.memset(ones64[:], 1.0)
                nc.vector.memset(nbias[:], -EXP8_SHIFT)

            def emit_kq(w_t, dst):
                # K^T/Q^T for a head pair: evict the [128, 512] PSUM
                # result straight to the top halves of the two Kd/Qd
                # tiles (partition-shifted copies), then duplicate each
                # top half into the bottom half with ONE whole-sequence
                # DMA per tile (the HWDGE costs ~fixed time per DMA, so
                # 2 big DMAs beat 32 small ones).
                for bt in range(ntb):
                    bs = slice(bt * nt4 * 128, (bt + 1) * nt4 * 128)
                    ps_t = ps_tile(f"{rp}kq{bt}_{w_t.name}")
                    for ch in range(CCH):
                        nc.tensor.matmul(
                            ps_t[:, 0:512], w_t[:, ch, 0:128], xT[:, bt, ch, :],
                            start=(ch == 0), stop=(ch == CCH - 1),
                        )
                    nc.any.tensor_copy(dst[0][0:64, bs], ps_t[0:64, 0:512])
                    nc.any.tensor_copy(dst[1][0:64, bs], ps_t[64:128, 0:512])
                nc.sync.dma_start(dst[0][64:128, :], dst[0][0:64, :])
                nc.sync.dma_start(dst[1][64:128, :], dst[1][0:64, :])

            def emit_v():
                # V for all heads: per token tile [tok, 3*64]
                for nt in range(nt_tiles):
                    bt, i = nt // nt4, nt % nt4
                    pv = ps_tile(f"{rp}pv{nt}")
                    for ch in range(CCH):
                        nc.tensor.matmul(
                            pv[:, 0:NH * D],
                            xT[:, bt, ch, i * 128:(i + 1) * 128], wv[:, ch, :],
                            start=(ch == 0), stop=(ch == CCH - 1),
                        )
                    nc.any.tensor_copy(
                        v_sb[:, :, nt, 0:64],
                        pv[:, 0:NH * D].rearrange("p (h d) -> p h d", h=NH))
                    nc.vector.tensor_tensor(
                        vr_sb[:, :, nt, 0:64],
                        pv[:, 0:NH * D].rearrange("p (h d) -> p h d", h=NH),
                        v_sb[:, :, nt, 0:64], mybir.AluOpType.subtract)

            # Heads 0/1 become ready first; KQ2 finishes during early
            # attention of head 0.
            emit_kq(wk, (Kd[0], Kd[1]))
            emit_kq(wq, (Qd[0], Qd[1]))
            emit_v()
            emit_kq(wkq2, (Kd[2], Qd[2]))

            # ---- phases 2+3: flash attention + interleaved projection ----
            ph3_jobs = []
            norm_jobs = []

            def emit_norm_job(job):
                # Deferred tail of softmax normalization: broadcast the
                # reciprocal rowsums via a tiny PE matmul and multiply.
                # Runs inside a later supertile stream so the rb matmul
                # never head-of-line-blocks the in-order PE behind a
                # staging+recip chain.
                qb_j, h, ou_sb, recip = job
                qs = slice(qb_j * qb, (qb_j + 1) * qb)
                dest = (outn01[0:64, qs], outn01[64:128, qs],
                        outn2d[0:64, qs])[h]
                rb = ps_tile(f"{rp}rb{qb_j}_{h}")
                for qi in range(qb // qh):
                    cs = slice(qi * qh, (qi + 1) * qh)
                    rb_h = rb[0:64, qi * qh:(qi + 1) * qh]
                    nc.tensor.matmul(rb_h, ones64[:], recip[:, cs],
                                     start=True, stop=True)
                    nc.vector.tensor_mul(dest[:, cs], ou_sb[0:64, cs], rb_h)
                if h == 2:
                    # duplicate head-2 rows for phase-3 row packing
                    nc.sync.dma_start(outn2d[64:128, qs], outn2d[0:64, qs])

            def emit_ph3_job(job):
                # One output-projection job: a full token tile x 768
                # cols in one PSUM slot ([*, 0:384] and [*, 512:896]).
                # The outn01 stationary is shared by both column chunks,
                # and the two K=64 head-2 matmuls run concurrently on
                # opposite PE row halves (outn2d/wp2d are duplicated).
                # One strided eviction + one contiguous output DMA.
                qb_j, nt = job
                ts = slice(nt * 128, (nt + 1) * 128)
                t = ps_tile(f"{rp}pp{nt}")
                ppA, ppB = t[:, 0:384], t[:, 512:896]
                nc.tensor.matmul(ppA, outn01[:, ts], wp01[:, 0:384],
                                 start=True, stop=False)
                nc.tensor.matmul(ppB, outn01[:, ts], wp01[:, 384:768],
                                 start=True, stop=False)
                nc.tensor.matmul(ppA, outn2d[0:64, ts], wp2d[0:64, 0:384],
                                 start=False, stop=True)
                nc.tensor.matmul(ppB, outn2d[64:128, ts], wp2d[64:128, 384:768],
                                 start=False, stop=True)
                po = pop.tile([128, 768], f16, tag="po", name=f"{rp}po{nt}")
                nc.any.tensor_copy(
                    po[:].rearrange("p (b k) -> p b k", b=2),
                    t[:].rearrange("p (b k) -> p b k", b=2)[:, :, 0:384])
                nc.sync.dma_start(out_dram[ts, :], po[:])

            for qb_i in range(nqb if phases >= 2 else 0):
                qs = slice(qb_i * qb, (qb_i + 1) * qb)
                for h in range(NH):
                    outT = accp.tile([65, qb], f32, tag="outT",
                                     name=f"{rp}outT{qb_i}_{h}")

                    def emit_pv_v(job):
                        # fp8 DoubleRow: one matmul contracts both k-tiles
                        # (256 keys) at 0.5 cycles/col: stationary
                        # [128, 2, 65] (two V planes), moving [128, 2, qh]
                        # (the P tile's two k-tile halves).
                        p, qi, pt = job
                        cs = slice(qi * qh, (qi + 1) * qh)
                        nc.tensor.matmul(
                            outT[:, cs], v_sb[:, h, 2 * p:2 * p + 2, 0:65],
                            pt[:].rearrange("p (t q) -> p t q", t=2),
                            start=(p == 0), stop=False,
                            perf_mode=DR,
                        )

                    def emit_pv_r(job):
                        # V-residual correction, staggered one supertile
                        # behind the v matmul so back-to-back writes to
                        # the same outT PSUM banks are spaced out (HW
                        # stalls on rapid same-bank revisits).
                        p, qi, pt = job
                        cs = slice(qi * qh, (qi + 1) * qh)
                        nc.tensor.matmul(
                            outT[:, cs], vr_sb[:, h, 2 * p:2 * p + 2, 0:65],
                            pt[:].rearrange("p (t q) -> p t q", t=2),
                            start=False, stop=(p == nkt // 2 - 1),
                            perf_mode=DR,
                        )

                    # Software-pipelined emission: each supertile's S
                    # matmuls + exp are emitted BEFORE the previous
                    # supertiles' PV matmuls (2-deep, so the first PVs
                    # of a head trail the previous head's accumulator
                    # staging by ~2 supertiles); one queued phase-3
                    # group is drained every 4 supertiles.
                    pending = []
                    last_v = [None]
                    for p in range(nkt // 2):
                        ktA, ktB = 2 * p, 2 * p + 1
                        for qi in range(qb // qh):
                            sidx = p * (qb // qh) + qi
                            if sidx % 4 == 3:
                                if norm_jobs:
                                    emit_norm_job(norm_jobs.pop(0))
                                elif ph3_jobs:
                                    emit_ph3_job(ph3_jobs.pop(0))
                            qcs = slice(qb_i * qb + qi * qh,
                                        qb_i * qb + (qi + 1) * qh)
                            st = sp_t = ps_tile(f"{rp}st{qb_i}_{h}_{p}_{qi}")
                            # back-to-back S matmuls on opposite PE row
                            # halves (concurrent via row tiling)
                            nc.tensor.matmul(
                                st[:, 0:qh],
                                Kd[h][0:64, ktA * 128:(ktA + 1) * 128],
                                Qd[h][0:64, qcs], start=True, stop=True,
                            )
                            nc.tensor.matmul(
                                st[:, qh:2 * qh],
                                Kd[h][64:128, ktB * 128:(ktB + 1) * 128],
                                Qd[h][64:128, qcs], start=True, stop=True,
                            )
                            pt = ptp.tile([128, 2 * qh], f8, tag="P",
                                          name=f"{rp}pt{qb_i}_{h}_{p}_{qi}")
                            if sidx % dve_den >= dve_den - dve_num:
                                # Schraudolph fast exp on DVE: one
                                # tensor_scalar producing fp8e4 exp bits
                                nc.vector.tensor_scalar(
                                    pt[:].bitcast(u8), st[:], EXP8_A, EXP8_B,
                                    mybir.AluOpType.mult, mybir.AluOpType.add)
                            else:
                                nc.scalar.activation(pt[:], st[:], Exp,
                                                     scale=SCALE,
                                                     bias=nbias[:])
                            pending.append((p, qi, pt))
                            if last_v[0] is not None:
                                emit_pv_r(last_v[0])
                                last_v[0] = None
                            if len(pending) > 3:
                                vjob = pending.pop(0)
                                emit_pv_v(vjob)
                                last_v[0] = vjob
                    if last_v[0] is not None:
                        emit_pv_r(last_v[0])
                        last_v[0] = None
                    for job in pending:
                        emit_pv_v(job)
                        emit_pv_r(job)

                    # Stage the accumulator to SBUF (frees the PSUM slot
                    # for the next head) and take the fp16 reciprocal of
                    # the rowsums; the broadcast+multiply is deferred
                    # into a later supertile stream (norm_jobs).
                    ou_sb = normp.tile([65, qb], f16, tag="ou",
                                       name=f"{rp}ou{qb_i}_{h}")
                    nc.any.tensor_copy(ou_sb[:], outT[:])
                    recip = normp.tile([1, qb], f16, tag="recip",
                                       name=f"{rp}rc{qb_i}_{h}")
                    with nc.allow_low_precision(reason="softmax recip"):
                        nc.vector.reciprocal(recip[:], ou_sb[64:65, :])
                    norm_jobs.append((qb_i, h, ou_sb, recip))

                if phases >= 3:
                    ph3_jobs += [(qb_i, qb_i * qb // 128 + j)
                                 for j in range(qb // 128)]
            while norm_jobs:
                emit_norm_job(norm_jobs.pop(0))
            while ph3_jobs:
                emit_ph3_job(ph3_jobs.pop(0))

    nc.compile()
    return nc


def get_nc(n_tokens=N, qb=1024, reps=1, phases=3, dve_den=11, dve_num=5):
    key = (n_tokens, qb, reps, phases, dve_den, dve_num)
    if key not in _CACHED_NC:
        _CACHED_NC[key] = _build(n_tokens, qb, reps, phases, dve_den, dve_num)
    return _CACHED_NC[key]


def _chunked(w):
    """[C, d] -> [128, CCH*d]: channel c = a*128 + p lands on partition p,
    chunk a (the on-chip [p, a, d] layout, flattened)."""
    d = w.shape[1]
    return np.ascontiguousarray(
        w.reshape(CCH, 128, d).transpose(1, 0, 2).reshape(128, CCH * d))


def make_in_maps(x, W_qkv, W_proj):
    """Per-core input dicts. Core c = b*4 + g: batch b, heads [3g, 3g+3).
    Host-side prep: fp16 cast + pre-arrangement to on-chip layouts."""
    f16 = np.float16
    x = np.asarray(x, np.float32)
    W_qkv = np.asarray(W_qkv, np.float32)
    W_proj = np.asarray(W_proj, np.float32)
    ntb = N // 512
    # x[b].T [C, N] -> [128, ntb*CCH*512] with index [p, (bt, a, col)]
    xT = [np.ascontiguousarray(
        x[b].T.reshape(CCH, 128, ntb, 512).transpose(1, 2, 0, 3)
        .reshape(128, -1)).astype(f16) for b in range(B)]
    in_maps = []
    for core in range(8):
        b, g = core // 4, core % 4
        h0 = g * NH * D  # column offset of this group's heads
        wk_s = W_qkv[:, C + h0:C + h0 + NH * D].astype(f16)
        wq_s = W_qkv[:, h0:h0 + NH * D].astype(f16)
        wp2 = W_proj[h0 + 128:h0 + 192].astype(f16)  # [64, C]
        in_maps.append({
            "xT_b": xT[b],
            "w_q": _chunked(wq_s),
            "w_k": _chunked(wk_s),
            "w_v": _chunked(
                W_qkv[:, 2 * C + h0:2 * C + h0 + NH * D].astype(f16)),
            "w_kq2": _chunked(
                np.concatenate([wk_s[:, 128:192], wq_s[:, 128:192]], axis=1)),
            "w_p01": np.ascontiguousarray(W_proj[h0:h0 + 128].astype(f16)),
            "w_p2d": np.ascontiguousarray(np.concatenate([wp2, wp2], axis=0)),
        })
    return in_maps


def kernel(x, W_qkv, W_proj, b_proj):
    from concourse.bass_utils import run_bass_kernel_spmd

    nc = get_nc()
    in_maps = make_in_maps(x, W_qkv, W_proj)
    res = run_bass_kernel_spmd(nc, in_maps, core_ids=list(range(8)))
    partials = [np.asarray(res.results[c]["out"], np.float32) for c in range(8)]
    out = np.stack([
        partials[0] + partials[1] + partials[2] + partials[3],
        partials[4] + partials[5] + partials[6] + partials[7],
    ])
    return (out + np.asarray(b_proj, np.float32)).astype(np.float32)



# revision 28
# speedup vs baseline: 1.0762x; 1.0762x over previous
"""Trainium2 Bass kernel for DiT multi-head attention block.

Computes, for x [B=2, N=4096, C=768]:
    qkv = x @ W_qkv                      # [B, N, 3C], no bias
    q, k, v = split(qkv) -> [B, H=12, N, D=64]
    attn = softmax(q k^T / sqrt(D))
    out  = (attn @ v) -> [B, N, C]
    out @ W_proj + b_proj

Sharding over 8 NeuronCores: core = b*4 + g handles batch b and the 3
heads [3g, 3g+3). Each core computes its heads' K/V/Q projections over
the full sequence, flash attention (no-max-subtraction softmax: scores
are ~N(0,1) so exp never overflows), and a partial output projection
restricted to its heads' rows of W_proj. The host sums the 4 fp16
partials per batch and adds the bias. No cross-core collectives.

v2 design notes (baseline was 685us, ACT-exp and pipeline-depth bound):
  - fp16 datapath everywhere (same PE speed as bf16, 8x finer
    quantization) frees error budget for the Schraudolph exp below.
  - The 50M-elem/core softmax exp is split between the ACT engine
    (table exp) and the DVE (a single tensor_scalar computing the
    Schraudolph bit-pattern of exp(s*SCALE)). ACT alone was a 400us
    serial bottleneck.
  - Software-pipelined emission: each supertile's S matmuls + exp are
    emitted before the previous supertile's PV, so S runs early on the
    in-order PE and the exp engines never starve.
  - K^T/Q^T half-duplication (for PE row-tiled S pairs) via SBUF->SBUF
    DMA; PSUM evictions on nc.any so the scheduler balances ACT/DVE.
  - Phase 3 (output projection) is interleaved into the next q-block's
    supertile stream one [128-token-pair x 384-col] group at a time.
    Output partials are written fp16.

v3 (514us -> ~450-550us depending on device day): fp8 P*V.
  - P (post-softmax probs) is produced directly in fp8e4m3: the ACT
    path writes fp8 from the exp table (with a -2.0 bias that cancels
    in the softmax ratio and keeps e4m3 bits clear of NaN for scores
    up to +8.2 sigma); the DVE path computes Schraudolph uint8 e4m3
    bits (uint8 conversion saturates at 0, so deep-negative scores
    become +0.0 rather than 0xFF).
  - V is stored fp8 and each PV step is ONE DoubleRow matmul
    contracting 256 keys ([128,2,65] stationary planes, the P tile's
    two k-tile halves as moving planes). Measured on HW: a DoubleRow
    matmul costs the same as an fp16 matmul of equal moving size, so
    the win is halved PE instruction count, not faster streaming.
  - fp8 V quantization error (~2.8% rel) breaks the 2e-2 gate, so a
    second accumulating DoubleRow matmul applies the quantization
    residual fp8(V - fp8(V)), staggered one supertile behind its v
    matmul to space out same-PSUM-bank writes (rapid same-bank
    revisits stall the PE; measured 185ns vs 1200ns per matmul).
  - Measured rel err 1.54e-2 (emulation-validated: P-e4m3 ~1.5%
    dominates; V residual compensation removes the V term).
"""

import numpy as np

B = 2
N = 4096
C = 768
H = 12
D = 64
SCALE = D ** -0.5
NH = 3  # heads per core
CCH = C // 128  # contract chunks over channels

# Schraudolph fp16-exp constants (DVE path): bits = round(s*A + B) with
# s the raw (unscaled) score; exp(s*SCALE) ~= bitcast_fp16(bits).
# c centers the multiplicative error (range +-3.9%) to ~zero mean.
EXP_A = (1 << 10) / float(np.log(2.0)) * SCALE
EXP_C = 57.0
EXP_B = float(15 * (1 << 10)) - EXP_C

# Schraudolph fp8e4m3-exp constants (DVE path, v3): uint8 bits of the
# e4m3 representation of exp(s*SCALE - EXP8_SHIFT):
# bits = round(s*A8 + B8). e4m3 bias 7, 3 mantissa bits; c8 = 57/128
# recenters like EXP_C. The SHIFT cancels exactly in the softmax ratio
# and keeps the e4m3 bits clear of the NaN boundary (bits=127) up to
# +8.2 sigma scores; uint8 conversion saturates at 0 (verified on HW),
# so deep-negative scores land at +0.0 instead of 0xFF (-NaN).
EXP8_SHIFT = 2.0
EXP8_A = (1 << 3) / float(np.log(2.0)) * SCALE
EXP8_C = 57.0 / 128.0
EXP8_B = float(7 * (1 << 3)) - EXP8_C - EXP8_SHIFT * (1 << 3) / float(np.log(2.0))

_CACHED_NC = {}


def _build(n_tokens=N, qb=1024, reps=1, phases=3, dve_den=11, dve_num=5):
    """reps > 1 repeats the whole computation inside one NEFF (timing
    aid: the walltime delta between reps and reps=1 isolates the
    on-device execution time from the multi-ms host dispatch cost).
    dve_num/dve_den: fraction of exp supertiles computed on the DVE via
    Schraudolph (rest on ACT)."""
    import concourse.bacc as bacc
    import concourse.bass as bass
    from concourse import mybir, tile

    f32 = mybir.dt.float32
    f16 = mybir.dt.float16
    i16 = mybir.dt.int16
    f8 = mybir.dt.float8e4
    u8 = mybir.dt.uint8
    DR = mybir.MatmulPerfMode.DoubleRow
    PSUM = bass.MemorySpace.PSUM
    Exp = mybir.ActivationFunctionType.Exp

    nt_tiles = n_tokens // 128   # token tiles (also k tiles)
    nqb = n_tokens // qb         # q blocks
    nkt = nt_tiles               # k tiles of 128
    qh = min(512, qb)            # q columns per matmul chunk
    nt4 = 4                      # token tiles per qkv batch
    ntb = nt_tiles // nt4        # qkv batches

    nc = bacc.Bacc("TRN2", target_bir_lowering=False, debug=False)

    # Inputs arrive pre-sharded, pre-cast to fp16, and pre-arranged to
    # the on-chip [partition, chunk, col] layouts so every load is one
    # contiguous DMA (no strided descriptor storm on the HWDGE).
    xT_dram = nc.declare_dram_parameter("xT_b", [128, ntb * CCH * nt4 * 128], f16, isOutput=False)
    wq_dram = nc.declare_dram_parameter("w_q", [128, CCH * NH * D], f16, isOutput=False)
    wk_dram = nc.declare_dram_parameter("w_k", [128, CCH * NH * D], f16, isOutput=False)
    wv_dram = nc.declare_dram_parameter("w_v", [128, CCH * NH * D], f16, isOutput=False)
    wkq2_dram = nc.declare_dram_parameter("w_kq2", [128, CCH * 128], f16, isOutput=False)
    wp01_dram = nc.declare_dram_parameter("w_p01", [128, C], f16, isOutput=False)
    wp2d_dram = nc.declare_dram_parameter("w_p2d", [128, C], f16, isOutput=False)
    out_dram = nc.declare_dram_parameter("out", [n_tokens, C], f16, isOutput=True)

    from contextlib import ExitStack

    with tile.TileContext(nc) as tc, ExitStack() as ctx:
        pp = ctx.enter_context(tc.tile_pool(name="persist", bufs=1))
        # xT layout: [partition, token-batch, chunk, col]
        xT = pp.tile([128, ntb, CCH, nt4 * 128], f16, name="xT")
        Kd = [pp.tile([128, n_tokens], f16, name=f"Kd{h}") for h in range(NH)]
        Qd = [pp.tile([128, n_tokens], f16, name=f"Qd{h}") for h in range(NH)]
        # fp8 V, padded to a 128-wide slot per (head, k-tile) so the
        # DoubleRow ldweights plane stride (128) is a multiple of 64
        # (walrus ISA constraint). vr_sb holds the fp8 quantization
        # residual fp8(V - fp8(V)); a second accumulating DoubleRow
        # matmul applies it, cutting the V-quant error ~20x.
        v_sb = pp.tile([128, NH, nkt, 128], f8, name="v_sb")
        vr_sb = pp.tile([128, NH, nkt, 128], f8, name="vr_sb")
        outn01 = pp.tile([128, n_tokens], f16, name="outn01")
        outn2d = pp.tile([128, n_tokens], f16, name="outn2d")
        wq = pp.tile([128, CCH, NH * D], f16, name="wq")
        wk = pp.tile([128, CCH, NH * D], f16, name="wk")
        wv = pp.tile([128, CCH, NH * D], f16, name="wv")
        wkq2 = pp.tile([128, CCH, 128], f16, name="wkq2")
        wp01 = pp.tile([128, C], f16, name="wp01")
        wp2d = pp.tile([128, C], f16, name="wp2d")
        ones64 = pp.tile([1, 64], f16, name="ones64")
        nbias = pp.tile([128, 1], f32, name="nbias")

        # One shared PSUM ring (3 x 2 banks) + the PV accumulator
        # (1 x 2 banks) = exactly 8 PSUM banks. Depth 3 on the S ring
        # breaks the exp->PV->S->exp serial cycle (ring 2 + a second
        # accumulator measured 581us vs 553us for this layout).
        psp = ctx.enter_context(tc.tile_pool(name="ps", bufs=3, space=PSUM))
        accp = ctx.enter_context(tc.tile_pool(name="accpsum", bufs=1, space=PSUM))
        ptp = ctx.enter_context(tc.tile_pool(name="ptile", bufs=10))
        normp = ctx.enter_context(tc.tile_pool(name="norm", bufs=2))
        rbp = ctx.enter_context(tc.tile_pool(name="rb", bufs=2))
        pop = ctx.enter_context(tc.tile_pool(name="projsb", bufs=6))

        def ps_tile(name):
            return psp.tile([128, 1024], f32, tag="S", name=name)

        # ---- weights: plain contiguous DMAs (pre-arranged host-side),
        # emitted in first-use order so phase 1 starts ASAP ----
        nc.sync.dma_start(wk[:].rearrange("p a d -> p (a d)"), wk_dram[:])
        nc.sync.dma_start(wq[:].rearrange("p a d -> p (a d)"), wq_dram[:])

        for rep in range(reps):
            rp = f"r{rep}_"

            # ---- phase 1: x load + qkv projections ----
            nbt = CCH * nt4 * 128  # fp16 elems per (partition, batch)
            for bt in range(ntb):
                nc.sync.dma_start(
                    xT[:, bt, :, :].rearrange("p a n -> p (a n)"),
                    xT_dram[:, bt * nbt:(bt + 1) * nbt])
            if rep == 0:
                # remaining weights, behind the first x batches
                nc.sync.dma_start(wv[:].rearrange("p a d -> p (a d)"), wv_dram[:])
                nc.sync.dma_start(wkq2[:].rearrange("p a d -> p (a d)"), wkq2_dram[:])
                nc.sync.dma_start(wp01[:], wp01_dram[:])
                nc.sync.dma_start(wp2d[:], wp2d_dram[:])
                nc.vector.memset(v_sb[:, :, :, 64:65], 1.0)
                nc.vector.memset(vr_sb[:, :, :, 64:65], 0.0)
                nc.vector.memset(ones64[:], 1.0)
                nc.vector.memset(nbias[:], -EXP8_SHIFT)

            def emit_kq(w_t, dst):
                # K^T/Q^T for a head pair: evict the [128, 512] PSUM
                # result straight to the top halves of the two Kd/Qd
                # tiles (partition-shifted copies), then duplicate each
                # top half into the bottom half with ONE whole-sequence
                # DMA per tile (the HWDGE costs ~fixed time per DMA, so
                # 2 big DMAs beat 32 small ones).
                for bt in range(ntb):
                    bs = slice(bt * nt4 * 128, (bt + 1) * nt4 * 128)
                    ps_t = ps_tile(f"{rp}kq{bt}_{w_t.name}")
                    for ch in range(CCH):
                        nc.tensor.matmul(
                            ps_t[:, 0:512], w_t[:, ch, 0:128], xT[:, bt, ch, :],
                            start=(ch == 0), stop=(ch == CCH - 1),
                        )
                    nc.any.tensor_copy(dst[0][0:64, bs], ps_t[0:64, 0:512])
                    nc.any.tensor_copy(dst[1][0:64, bs], ps_t[64:128, 0:512])
                nc.sync.dma_start(dst[0][64:128, :], dst[0][0:64, :])
                nc.sync.dma_start(dst[1][64:128, :], dst[1][0:64, :])

            def emit_v():
                # V for all heads: per token tile [tok, 3*64]
                for nt in range(nt_tiles):
                    bt, i = nt // nt4, nt % nt4
                    pv = ps_tile(f"{rp}pv{nt}")
                    for ch in range(CCH):
                        nc.tensor.matmul(
                            pv[:, 0:NH * D],
                            xT[:, bt, ch, i * 128:(i + 1) * 128], wv[:, ch, :],
                            start=(ch == 0), stop=(ch == CCH - 1),
                        )
                    nc.any.tensor_copy(
                        v_sb[:, :, nt, 0:64],
                        pv[:, 0:NH * D].rearrange("p (h d) -> p h d", h=NH))
                    nc.vector.tensor_tensor(
                        vr_sb[:, :, nt, 0:64],
                        pv[:, 0:NH * D].rearrange("p (h d) -> p h d", h=NH),
                        v_sb[:, :, nt, 0:64], mybir.AluOpType.subtract)

            # Heads 0/1 become ready first; KQ2 finishes during early
            # attention of head 0.
            emit_kq(wk, (Kd[0], Kd[1]))
            emit_kq(wq, (Qd[0], Qd[1]))
            emit_v()
            emit_kq(wkq2, (Kd[2], Qd[2]))

            # ---- phases 2+3: flash attention + interleaved projection ----
            ph3_jobs = []
            norm_jobs = []

            def emit_norm_job(job):
                # Deferred tail of softmax normalization: broadcast the
                # reciprocal rowsums to 64 partitions on the otherwise
                # idle GPSIMD engine (frees the PE matmuls + PSUM ring
                # slots the old ones-matmul broadcast used), then
                # multiply on DVE.
                qb_j, h, ou_sb, recip = job
                qs = slice(qb_j * qb, (qb_j + 1) * qb)
                dest = (outn01[0:64, qs], outn01[64:128, qs],
                        outn2d[0:64, qs])[h]
                rb_sb = rbp.tile([64, qb], f16, tag="rb",
                                 name=f"{rp}rb{qb_j}_{h}")
                nc.gpsimd.partition_broadcast(rb_sb[:], recip[:], channels=64)
                nc.vector.tensor_mul(dest, ou_sb[0:64, :], rb_sb[:])
                if h == 2:
                    # duplicate head-2 rows for phase-3 row packing
                    nc.sync.dma_start(outn2d[64:128, qs], outn2d[0:64, qs])

            def emit_ph3_job(job):
                # One output-projection job: a full token tile x 768
                # cols in one PSUM slot ([*, 0:384] and [*, 512:896]).
                # The outn01 stationary is shared by both column chunks,
                # and the two K=64 head-2 matmuls run concurrently on
                # opposite PE row halves (outn2d/wp2d are duplicated).
                # One strided eviction + one contiguous output DMA.
                qb_j, nt = job
                ts = slice(nt * 128, (nt + 1) * 128)
                t = ps_tile(f"{rp}pp{nt}")
                ppA, ppB = t[:, 0:384], t[:, 512:896]
                nc.tensor.matmul(ppA, outn01[:, ts], wp01[:, 0:384],
                                 start=True, stop=False)
                nc.tensor.matmul(ppB, outn01[:, ts], wp01[:, 384:768],
                                 start=True, stop=False)
                nc.tensor.matmul(ppA, outn2d[0:64, ts], wp2d[0:64, 0:384],
                                 start=False, stop=True)
                nc.tensor.matmul(ppB, outn2d[64:128, ts], wp2d[64:128, 384:768],
                                 start=False, stop=True)
                po = pop.tile([128, 768], f16, tag="po", name=f"{rp}po{nt}")
                nc.any.tensor_copy(
                    po[:].rearrange("p (b k) -> p b k", b=2),
                    t[:].rearrange("p (b k) -> p b k", b=2)[:, :, 0:384])
                nc.sync.dma_start(out_dram[ts, :], po[:])

            for qb_i in range(nqb if phases >= 2 else 0):
                qs = slice(qb_i * qb, (qb_i + 1) * qb)
                for h in range(NH):
                    outT = accp.tile([65, qb], f32, tag="outT",
                                     name=f"{rp}outT{qb_i}_{h}")

                    def emit_pv_v(job):
                        # fp8 DoubleRow: one matmul contracts both k-tiles
                        # (256 keys) at 0.5 cycles/col: stationary
                        # [128, 2, 65] (two V planes), moving [128, 2, qh]
                        # (the P tile's two k-tile halves).
                        p, qi, pt = job
                        cs = slice(qi * qh, (qi + 1) * qh)
                        nc.tensor.matmul(
                            outT[:, cs], v_sb[:, h, 2 * p:2 * p + 2, 0:65],
                            pt[:].rearrange("p (t q) -> p t q", t=2),
                            start=(p == 0), stop=False,
                            perf_mode=DR,
                        )

                    def emit_pv_r(job):
                        # V-residual correction, staggered one supertile
                        # behind the v matmul so back-to-back writes to
                        # the same outT PSUM banks are spaced out (HW
                        # stalls on rapid same-bank revisits).
                        p, qi, pt = job
                        cs = slice(qi * qh, (qi + 1) * qh)
                        nc.tensor.matmul(
                            outT[:, cs], vr_sb[:, h, 2 * p:2 * p + 2, 0:65],
                            pt[:].rearrange("p (t q) -> p t q", t=2),
                            start=False, stop=(p == nkt // 2 - 1),
                            perf_mode=DR,
                        )

                    # Software-pipelined emission: each supertile's S
                    # matmuls + exp are emitted BEFORE the previous
                    # supertiles' PV matmuls (2-deep, so the first PVs
                    # of a head trail the previous head's accumulator
                    # staging by ~2 supertiles); one queued phase-3
                    # group is drained every 4 supertiles.
                    pending = []
                    last_v = [None]
                    for p in range(nkt // 2):
                        ktA, ktB = 2 * p, 2 * p + 1
                        for qi in range(qb // qh):
                            sidx = p * (qb // qh) + qi
                            if sidx % 4 == 3:
                                if norm_jobs:
                                    emit_norm_job(norm_jobs.pop(0))
                                elif ph3_jobs:
                                    emit_ph3_job(ph3_jobs.pop(0))
                            qcs = slice(qb_i * qb + qi * qh,
                                        qb_i * qb + (qi + 1) * qh)
                            st = sp_t = ps_tile(f"{rp}st{qb_i}_{h}_{p}_{qi}")
                            # back-to-back S matmuls on opposite PE row
                            # halves (concurrent via row tiling)
                            nc.tensor.matmul(
                                st[:, 0:qh],
                                Kd[h][0:64, ktA * 128:(ktA + 1) * 128],
                                Qd[h][0:64, qcs], start=True, stop=True,
                            )
                            nc.tensor.matmul(
                                st[:, qh:2 * qh],
                                Kd[h][64:128, ktB * 128:(ktB + 1) * 128],
                                Qd[h][64:128, qcs], start=True, stop=True,
                            )
                            pt = ptp.tile([128, 2 * qh], f8, tag="P",
                                          name=f"{rp}pt{qb_i}_{h}_{p}_{qi}")
                            if sidx % dve_den >= dve_den - dve_num:
                                # Schraudolph fast exp on DVE: one
                                # tensor_scalar producing fp8e4 exp bits
                                nc.vector.tensor_scalar(
                                    pt[:].bitcast(u8), st[:], EXP8_A, EXP8_B,
                                    mybir.AluOpType.mult, mybir.AluOpType.add)
                            else:
                                nc.scalar.activation(pt[:], st[:], Exp,
                                                     scale=SCALE,
                                                     bias=nbias[:])
                            pending.append((p, qi, pt))
                            if last_v[0] is not None:
                                emit_pv_r(last_v[0])
                                last_v[0] = None
                            if len(pending) > 3:
                                vjob = pending.pop(0)
                                emit_pv_v(vjob)
                                last_v[0] = vjob
                    if last_v[0] is not None:
                        emit_pv_r(last_v[0])
                        last_v[0] = None
                    for job in pending:
                        emit_pv_v(job)
                        emit_pv_r(job)

                    # Stage the accumulator to SBUF (frees the PSUM slot
                    # for the next head) and take the fp16 reciprocal of
                    # the rowsums; the broadcast+multiply is deferred
                    # into a later supertile stream (norm_jobs).
                    ou_sb = normp.tile([65, qb], f16, tag="ou",
                                       name=f"{rp}ou{qb_i}_{h}")
                    nc.any.tensor_copy(ou_sb[:], outT[:])
                    recip = normp.tile([1, qb], f16, tag="recip",
                                       name=f"{rp}rc{qb_i}_{h}")
                    with nc.allow_low_precision(reason="softmax recip"):
                        nc.vector.reciprocal(recip[:], ou_sb[64:65, :])
                    norm_jobs.append((qb_i, h, ou_sb, recip))

                if phases >= 3:
                    ph3_jobs += [(qb_i, qb_i * qb // 128 + j)
                                 for j in range(qb // 128)]
            while norm_jobs:
                emit_norm_job(norm_jobs.pop(0))
            while ph3_jobs:
                emit_ph3_job(ph3_jobs.pop(0))

    nc.compile()
    return nc


def get_nc(n_tokens=N, qb=1024, reps=1, phases=3, dve_den=11, dve_num=5):
    key = (n_tokens, qb, reps, phases, dve_den, dve_num)
    if key not in _CACHED_NC:
        _CACHED_NC[key] = _build(n_tokens, qb, reps, phases, dve_den, dve_num)
    return _CACHED_NC[key]


def _chunked(w):
    """[C, d] -> [128, CCH*d]: channel c = a*128 + p lands on partition p,
    chunk a (the on-chip [p, a, d] layout, flattened)."""
    d = w.shape[1]
    return np.ascontiguousarray(
        w.reshape(CCH, 128, d).transpose(1, 0, 2).reshape(128, CCH * d))


def make_in_maps(x, W_qkv, W_proj):
    """Per-core input dicts. Core c = b*4 + g: batch b, heads [3g, 3g+3).
    Host-side prep: fp16 cast + pre-arrangement to on-chip layouts."""
    f16 = np.float16
    x = np.asarray(x, np.float32)
    W_qkv = np.asarray(W_qkv, np.float32)
    W_proj = np.asarray(W_proj, np.float32)
    ntb = N // 512
    # x[b].T [C, N] -> [128, ntb*CCH*512] with index [p, (bt, a, col)]
    xT = [np.ascontiguousarray(
        x[b].T.reshape(CCH, 128, ntb, 512).transpose(1, 2, 0, 3)
        .reshape(128, -1)).astype(f16) for b in range(B)]
    in_maps = []
    for core in range(8):
        b, g = core // 4, core % 4
        h0 = g * NH * D  # column offset of this group's heads
        wk_s = W_qkv[:, C + h0:C + h0 + NH * D].astype(f16)
        wq_s = W_qkv[:, h0:h0 + NH * D].astype(f16)
        wp2 = W_proj[h0 + 128:h0 + 192].astype(f16)  # [64, C]
        in_maps.append({
            "xT_b": xT[b],
            "w_q": _chunked(wq_s),
            "w_k": _chunked(wk_s),
            "w_v": _chunked(
                W_qkv[:, 2 * C + h0:2 * C + h0 + NH * D].astype(f16)),
            "w_kq2": _chunked(
                np.concatenate([wk_s[:, 128:192], wq_s[:, 128:192]], axis=1)),
            "w_p01": np.ascontiguousarray(W_proj[h0:h0 + 128].astype(f16)),
            "w_p2d": np.ascontiguousarray(np.concatenate([wp2, wp2], axis=0)),
        })
    return in_maps


def kernel(x, W_qkv, W_proj, b_proj):
    from concourse.bass_utils import run_bass_kernel_spmd

    nc = get_nc()
    in_maps = make_in_maps(x, W_qkv, W_proj)
    res = run_bass_kernel_spmd(nc, in_maps, core_ids=list(range(8)))
    partials = [np.asarray(res.results[c]["out"], np.float32) for c in range(8)]
    out = np.stack([
        partials[0] + partials[1] + partials[2] + partials[3],
        partials[4] + partials[5] + partials[6] + partials[7],
    ])
    return (out + np.asarray(b_proj, np.float32)).astype(np.float32)



# revision 30
# speedup vs baseline: 1.2137x; 1.1278x over previous
"""Trainium2 Bass kernel for DiT multi-head attention block.

Computes, for x [B=2, N=4096, C=768]:
    qkv = x @ W_qkv                      # [B, N, 3C], no bias
    q, k, v = split(qkv) -> [B, H=12, N, D=64]
    attn = softmax(q k^T / sqrt(D))
    out  = (attn @ v) -> [B, N, C]
    out @ W_proj + b_proj

Sharding over 8 NeuronCores: core = b*4 + g handles batch b and the 3
heads [3g, 3g+3). Each core computes its heads' K/V/Q projections over
the full sequence, flash attention (no-max-subtraction softmax: scores
are ~N(0,1) so exp never overflows), and a partial output projection
restricted to its heads' rows of W_proj. The host sums the 4 fp16
partials per batch and adds the bias. No cross-core collectives.

v2 design notes (baseline was 685us, ACT-exp and pipeline-depth bound):
  - fp16 datapath everywhere (same PE speed as bf16, 8x finer
    quantization) frees error budget for the Schraudolph exp below.
  - The 50M-elem/core softmax exp is split between the ACT engine
    (table exp) and the DVE (a single tensor_scalar computing the
    Schraudolph bit-pattern of exp(s*SCALE)). ACT alone was a 400us
    serial bottleneck.
  - Software-pipelined emission: each supertile's S matmuls + exp are
    emitted before the previous supertile's PV, so S runs early on the
    in-order PE and the exp engines never starve.
  - K^T/Q^T half-duplication (for PE row-tiled S pairs) via SBUF->SBUF
    DMA; PSUM evictions on nc.any so the scheduler balances ACT/DVE.
  - Phase 3 (output projection) is interleaved into the next q-block's
    supertile stream one [128-token-pair x 384-col] group at a time.
    Output partials are written fp16.

v3 (514us -> ~450-550us depending on device day): fp8 P*V.
  - P (post-softmax probs) is produced directly in fp8e4m3: the ACT
    path writes fp8 from the exp table (with a -2.0 bias that cancels
    in the softmax ratio and keeps e4m3 bits clear of NaN for scores
    up to +8.2 sigma); the DVE path computes Schraudolph uint8 e4m3
    bits (uint8 conversion saturates at 0, so deep-negative scores
    become +0.0 rather than 0xFF).
  - V is stored fp8 and each PV step is ONE DoubleRow matmul
    contracting 256 keys ([128,2,65] stationary planes, the P tile's
    two k-tile halves as moving planes). Measured on HW: a DoubleRow
    matmul costs the same as an fp16 matmul of equal moving size, so
    the win is halved PE instruction count, not faster streaming.
  - fp8 V quantization error (~2.8% rel) breaks the 2e-2 gate, so a
    second accumulating DoubleRow matmul applies the quantization
    residual fp8(V - fp8(V)), staggered one supertile behind its v
    matmul to space out same-PSUM-bank writes (rapid same-bank
    revisits stall the PE; measured 185ns vs 1200ns per matmul).
  - Measured rel err 1.54e-2 (emulation-validated: P-e4m3 ~1.5%
    dominates; V residual compensation removes the V term).
"""

import numpy as np

B = 2
N = 4096
C = 768
H = 12
D = 64
SCALE = D ** -0.5
NH = 3  # heads per core
CCH = C // 128  # contract chunks over channels

# Schraudolph fp16-exp constants (DVE path): bits = round(s*A + B) with
# s the raw (unscaled) score; exp(s*SCALE) ~= bitcast_fp16(bits).
# c centers the multiplicative error (range +-3.9%) to ~zero mean.
EXP_A = (1 << 10) / float(np.log(2.0)) * SCALE
EXP_C = 57.0
EXP_B = float(15 * (1 << 10)) - EXP_C

# Schraudolph fp8e4m3-exp constants (DVE path, v3): uint8 bits of the
# e4m3 representation of exp(s*SCALE - EXP8_SHIFT):
# bits = round(s*A8 + B8). e4m3 bias 7, 3 mantissa bits; c8 = 57/128
# recenters like EXP_C. The SHIFT cancels exactly in the softmax ratio
# and keeps the e4m3 bits clear of the NaN boundary (bits=127) up to
# +8.2 sigma scores; uint8 conversion saturates at 0 (verified on HW),
# so deep-negative scores land at +0.0 instead of 0xFF (-NaN).
EXP8_SHIFT = 2.0
EXP8_A = (1 << 3) / float(np.log(2.0)) * SCALE
EXP8_C = 57.0 / 128.0
EXP8_B = float(7 * (1 << 3)) - EXP8_C - EXP8_SHIFT * (1 << 3) / float(np.log(2.0))

_CACHED_NC = {}


def _build(n_tokens=N, qb=1024, reps=1, phases=3, dve_den=11, dve_num=5):
    """reps > 1 repeats the whole computation inside one NEFF (timing
    aid: the walltime delta between reps and reps=1 isolates the
    on-device execution time from the multi-ms host dispatch cost).
    dve_num/dve_den: fraction of exp supertiles computed on the DVE via
    Schraudolph (rest on ACT)."""
    import concourse.bacc as bacc
    import concourse.bass as bass
    from concourse import mybir, tile

    f32 = mybir.dt.float32
    f16 = mybir.dt.float16
    i16 = mybir.dt.int16
    f8 = mybir.dt.float8e4
    u8 = mybir.dt.uint8
    DR = mybir.MatmulPerfMode.DoubleRow
    PSUM = bass.MemorySpace.PSUM
    Exp = mybir.ActivationFunctionType.Exp

    nt_tiles = n_tokens // 128   # token tiles (also k tiles)
    nqb = n_tokens // qb         # q blocks
    nkt = nt_tiles               # k tiles of 128
    qh = min(512, qb)            # q columns per matmul chunk
    nt4 = 4                      # token tiles per qkv batch
    ntb = nt_tiles // nt4        # qkv batches

    nc = bacc.Bacc("TRN2", target_bir_lowering=False, debug=False)

    # Inputs arrive pre-sharded, pre-cast to fp16, and pre-arranged to
    # the on-chip [partition, chunk, col] layouts so every load is one
    # contiguous DMA (no strided descriptor storm on the HWDGE).
    xT_dram = nc.declare_dram_parameter("xT_b", [128, ntb * CCH * nt4 * 128], f16, isOutput=False)
    wq_dram = nc.declare_dram_parameter("w_q", [128, CCH * NH * D], f16, isOutput=False)
    wk_dram = nc.declare_dram_parameter("w_k", [128, CCH * NH * D], f16, isOutput=False)
    wv_dram = nc.declare_dram_parameter("w_v", [128, CCH * NH * D], f16, isOutput=False)
    wkq2_dram = nc.declare_dram_parameter("w_kq2", [128, CCH * 128], f16, isOutput=False)
    wp01_dram = nc.declare_dram_parameter("w_p01", [128, C], f16, isOutput=False)
    wp2d_dram = nc.declare_dram_parameter("w_p2d", [128, C], f16, isOutput=False)
    out_dram = nc.declare_dram_parameter("out", [n_tokens, C], f16, isOutput=True)

    from contextlib import ExitStack

    with tile.TileContext(nc) as tc, ExitStack() as ctx:
        pp = ctx.enter_context(tc.tile_pool(name="persist", bufs=1))
        # xT layout: [partition, token-batch, chunk, col]
        xT = pp.tile([128, ntb, CCH, nt4 * 128], f16, name="xT")
        Kd = [pp.tile([128, n_tokens], f16, name=f"Kd{h}") for h in range(NH)]
        Qd = [pp.tile([128, n_tokens], f16, name=f"Qd{h}") for h in range(NH)]
        # fp8 V, padded to a 128-wide slot per (head, k-tile) so the
        # DoubleRow ldweights plane stride (128) is a multiple of 64
        # (walrus ISA constraint). vr_sb holds the fp8 quantization
        # residual fp8(V - fp8(V)); a second accumulating DoubleRow
        # matmul applies it, cutting the V-quant error ~20x.
        v_sb = pp.tile([128, NH, nkt, 128], f8, name="v_sb")
        vr_sb = pp.tile([128, NH, nkt, 128], f8, name="vr_sb")
        outn01 = pp.tile([128, n_tokens], f16, name="outn01")
        outn2d = pp.tile([128, n_tokens], f16, name="outn2d")
        wq = pp.tile([128, CCH, NH * D], f16, name="wq")
        wk = pp.tile([128, CCH, NH * D], f16, name="wk")
        wv = pp.tile([128, CCH, NH * D], f16, name="wv")
        wkq2 = pp.tile([128, CCH, 128], f16, name="wkq2")
        wp01 = pp.tile([128, C], f16, name="wp01")
        wp2d = pp.tile([128, C], f16, name="wp2d")
        ones64 = pp.tile([1, 64], f16, name="ones64")
        nbias = pp.tile([128, 1], f32, name="nbias")

        # One shared PSUM ring (3 x 2 banks) + the PV accumulator
        # (1 x 2 banks) = exactly 8 PSUM banks. Depth 3 on the S ring
        # breaks the exp->PV->S->exp serial cycle (ring 2 + a second
        # accumulator measured 581us vs 553us for this layout).
        psp = ctx.enter_context(tc.tile_pool(name="ps", bufs=3, space=PSUM))
        accp = ctx.enter_context(tc.tile_pool(name="accpsum", bufs=1, space=PSUM))
        ptp = ctx.enter_context(tc.tile_pool(name="ptile", bufs=10))
        normp = ctx.enter_context(tc.tile_pool(name="norm", bufs=2))
        rbp = ctx.enter_context(tc.tile_pool(name="rb", bufs=2))
        pop = ctx.enter_context(tc.tile_pool(name="projsb", bufs=6))

        def ps_tile(name):
            return psp.tile([128, 1024], f32, tag="S", name=name)

        # ---- weights: plain contiguous DMAs (pre-arranged host-side),
        # emitted in first-use order so phase 1 starts ASAP ----
        nc.sync.dma_start(wk[:].rearrange("p a d -> p (a d)"), wk_dram[:])
        nc.sync.dma_start(wq[:].rearrange("p a d -> p (a d)"), wq_dram[:])

        for rep in range(reps):
            rp = f"r{rep}_"

            # ---- phase 1: x load + qkv projections ----
            nbt = CCH * nt4 * 128  # fp16 elems per (partition, batch)
            for bt in range(ntb):
                nc.sync.dma_start(
                    xT[:, bt, :, :].rearrange("p a n -> p (a n)"),
                    xT_dram[:, bt * nbt:(bt + 1) * nbt])
            if rep == 0:
                # remaining weights, behind the first x batches
                nc.sync.dma_start(wv[:].rearrange("p a d -> p (a d)"), wv_dram[:])
                nc.sync.dma_start(wkq2[:].rearrange("p a d -> p (a d)"), wkq2_dram[:])
                nc.sync.dma_start(wp01[:], wp01_dram[:])
                nc.sync.dma_start(wp2d[:], wp2d_dram[:])
                nc.vector.memset(v_sb[:, :, :, 64:65], 1.0)
                nc.vector.memset(vr_sb[:, :, :, 64:65], 0.0)
                nc.vector.memset(ones64[:], 1.0)
                nc.vector.memset(nbias[:], -EXP8_SHIFT)

            def emit_kq(w_t, dst):
                # K^T/Q^T for a head pair: evict the [128, 512] PSUM
                # result straight to the top halves of the two Kd/Qd
                # tiles (partition-shifted copies), then duplicate each
                # top half into the bottom half with ONE whole-sequence
                # DMA per tile (the HWDGE costs ~fixed time per DMA, so
                # 2 big DMAs beat 32 small ones).
                for bt in range(ntb):
                    bs = slice(bt * nt4 * 128, (bt + 1) * nt4 * 128)
                    ps_t = ps_tile(f"{rp}kq{bt}_{w_t.name}")
                    for ch in range(CCH):
                        nc.tensor.matmul(
                            ps_t[:, 0:512], w_t[:, ch, 0:128], xT[:, bt, ch, :],
                            start=(ch == 0), stop=(ch == CCH - 1),
                        )
                    nc.any.tensor_copy(dst[0][0:64, bs], ps_t[0:64, 0:512])
                    nc.any.tensor_copy(dst[1][0:64, bs], ps_t[64:128, 0:512])
                nc.sync.dma_start(dst[0][64:128, :], dst[0][0:64, :])
                nc.sync.dma_start(dst[1][64:128, :], dst[1][0:64, :])

            def emit_v():
                # V for all heads: per token tile [tok, 3*64]
                for nt in range(nt_tiles):
                    bt, i = nt // nt4, nt % nt4
                    pv = ps_tile(f"{rp}pv{nt}")
                    for ch in range(CCH):
                        nc.tensor.matmul(
                            pv[:, 0:NH * D],
                            xT[:, bt, ch, i * 128:(i + 1) * 128], wv[:, ch, :],
                            start=(ch == 0), stop=(ch == CCH - 1),
                        )
                    nc.any.tensor_copy(
                        v_sb[:, :, nt, 0:64],
                        pv[:, 0:NH * D].rearrange("p (h d) -> p h d", h=NH))
                    nc.vector.tensor_tensor(
                        vr_sb[:, :, nt, 0:64],
                        pv[:, 0:NH * D].rearrange("p (h d) -> p h d", h=NH),
                        v_sb[:, :, nt, 0:64], mybir.AluOpType.subtract)

            # Heads 0/1 become ready first; KQ2 finishes during early
            # attention of head 0.
            emit_kq(wk, (Kd[0], Kd[1]))
            emit_kq(wq, (Qd[0], Qd[1]))
            emit_v()
            emit_kq(wkq2, (Kd[2], Qd[2]))

            # ---- phases 2+3: flash attention + interleaved projection ----
            ph3_jobs = []
            norm_jobs = []

            def emit_norm_job(job):
                # Deferred tail of softmax normalization: broadcast the
                # reciprocal rowsums to 64 partitions on the otherwise
                # idle GPSIMD engine (frees the PE matmuls + PSUM ring
                # slots the old ones-matmul broadcast used), then
                # multiply on DVE.
                qb_j, h, ou_sb, recip = job
                qs = slice(qb_j * qb, (qb_j + 1) * qb)
                dest = (outn01[0:64, qs], outn01[64:128, qs],
                        outn2d[0:64, qs])[h]
                rb_sb = rbp.tile([64, qb], f16, tag="rb",
                                 name=f"{rp}rb{qb_j}_{h}")
                nc.gpsimd.partition_broadcast(rb_sb[:], recip[:], channels=64)
                nc.vector.tensor_mul(dest, ou_sb[0:64, :], rb_sb[:])
                if h == 2:
                    # duplicate head-2 rows for phase-3 row packing
                    nc.sync.dma_start(outn2d[64:128, qs], outn2d[0:64, qs])

            def emit_ph3_job(job):
                # One output-projection job: a full token tile x 768
                # cols in one PSUM slot ([*, 0:384] and [*, 512:896]).
                # The outn01 stationary is shared by both column chunks,
                # and the two K=64 head-2 matmuls run concurrently on
                # opposite PE row halves (outn2d/wp2d are duplicated).
                # One strided eviction + one contiguous output DMA.
                qb_j, nt = job
                ts = slice(nt * 128, (nt + 1) * 128)
                t = ps_tile(f"{rp}pp{nt}")
                ppA, ppB = t[:, 0:384], t[:, 512:896]
                nc.tensor.matmul(ppA, outn01[:, ts], wp01[:, 0:384],
                                 start=True, stop=False)
                nc.tensor.matmul(ppB, outn01[:, ts], wp01[:, 384:768],
                                 start=True, stop=False)
                nc.tensor.matmul(ppA, outn2d[0:64, ts], wp2d[0:64, 0:384],
                                 start=False, stop=True)
                nc.tensor.matmul(ppB, outn2d[64:128, ts], wp2d[64:128, 384:768],
                                 start=False, stop=True)
                po = pop.tile([128, 768], f16, tag="po", name=f"{rp}po{nt}")
                nc.any.tensor_copy(
                    po[:].rearrange("p (b k) -> p b k", b=2),
                    t[:].rearrange("p (b k) -> p b k", b=2)[:, :, 0:384])
                nc.sync.dma_start(out_dram[ts, :], po[:])

            for qb_i in range(nqb if phases >= 2 else 0):
                qs = slice(qb_i * qb, (qb_i + 1) * qb)
                for h in range(NH):
                    outT = accp.tile([65, qb], f32, tag="outT",
                                     name=f"{rp}outT{qb_i}_{h}")

                    def emit_pv_v(job):
                        # fp8 DoubleRow: one matmul contracts both k-tiles
                        # (256 keys) at 0.5 cycles/col: stationary
                        # [128, 2, 65] (two V planes), moving [128, 2, qh]
                        # (the P tile's two k-tile halves).
                        p, qi, pt = job
                        cs = slice(qi * qh, (qi + 1) * qh)
                        nc.tensor.matmul(
                            outT[:, cs], v_sb[:, h, 2 * p:2 * p + 2, 0:65],
                            pt[:].rearrange("p (t q) -> p t q", t=2),
                            start=(p == 0), stop=(p == nkt // 2 - 1),
                            perf_mode=DR,
                        )

                    def emit_pv_r(job):
                        # V-residual correction, staggered one supertile
                        # behind the v matmul so back-to-back writes to
                        # the same outT PSUM banks are spaced out (HW
                        # stalls on rapid same-bank revisits). Applied to
                        # every other k-tile pair only: the V-quant error
                        # scales with sqrt(uncompensated keys), measured
                        # 1.84e-2 total at half compensation (vs 1.54e-2
                        # full, 2.10e-2 none) for half the residual mms.
                        p, qi, pt = job
                        if p % 2 != 0:
                            return
                        cs = slice(qi * qh, (qi + 1) * qh)
                        nc.tensor.matmul(
                            outT[:, cs], vr_sb[:, h, 2 * p:2 * p + 2, 0:65],
                            pt[:].rearrange("p (t q) -> p t q", t=2),
                            start=False, stop=False,
                            perf_mode=DR,
                        )

                    # Software-pipelined emission: each supertile's S
                    # matmuls + exp are emitted BEFORE the previous
                    # supertiles' PV matmuls (2-deep, so the first PVs
                    # of a head trail the previous head's accumulator
                    # staging by ~2 supertiles); one queued phase-3
                    # group is drained every 4 supertiles.
                    pending = []
                    last_v = [None]
                    for p in range(nkt // 2):
                        ktA, ktB = 2 * p, 2 * p + 1
                        for qi in range(qb // qh):
                            sidx = p * (qb // qh) + qi
                            if sidx % 4 == 3:
                                if norm_jobs:
                                    emit_norm_job(norm_jobs.pop(0))
                                elif ph3_jobs:
                                    emit_ph3_job(ph3_jobs.pop(0))
                            qcs = slice(qb_i * qb + qi * qh,
                                        qb_i * qb + (qi + 1) * qh)
                            st = sp_t = ps_tile(f"{rp}st{qb_i}_{h}_{p}_{qi}")
                            # back-to-back S matmuls on opposite PE row
                            # halves (concurrent via row tiling)
                            nc.tensor.matmul(
                                st[:, 0:qh],
                                Kd[h][0:64, ktA * 128:(ktA + 1) * 128],
                                Qd[h][0:64, qcs], start=True, stop=True,
                            )
                            nc.tensor.matmul(
                                st[:, qh:2 * qh],
                                Kd[h][64:128, ktB * 128:(ktB + 1) * 128],
                                Qd[h][64:128, qcs], start=True, stop=True,
                            )
                            pt = ptp.tile([128, 2 * qh], f8, tag="P",
                                          name=f"{rp}pt{qb_i}_{h}_{p}_{qi}")
                            if sidx % dve_den >= dve_den - dve_num:
                                # Schraudolph fast exp on DVE: one
                                # tensor_scalar producing fp8e4 exp bits
                                nc.vector.tensor_scalar(
                                    pt[:].bitcast(u8), st[:], EXP8_A, EXP8_B,
                                    mybir.AluOpType.mult, mybir.AluOpType.add)
                            else:
                                nc.scalar.activation(pt[:], st[:], Exp,
                                                     scale=SCALE,
                                                     bias=nbias[:])
                            pending.append((p, qi, pt))
                            if last_v[0] is not None:
                                emit_pv_r(last_v[0])
                                last_v[0] = None
                            if len(pending) > 3:
                                vjob = pending.pop(0)
                                emit_pv_v(vjob)
                                last_v[0] = vjob
                    if last_v[0] is not None:
                        emit_pv_r(last_v[0])
                        last_v[0] = None
                    for job in pending:
                        emit_pv_v(job)
                        emit_pv_r(job)

                    # Stage the accumulator to SBUF (frees the PSUM slot
                    # for the next head) and take the fp16 reciprocal of
                    # the rowsums; the broadcast+multiply is deferred
                    # into a later supertile stream (norm_jobs).
                    ou_sb = normp.tile([65, qb], f16, tag="ou",
                                       name=f"{rp}ou{qb_i}_{h}")
                    nc.any.tensor_copy(ou_sb[:], outT[:])
                    recip = normp.tile([1, qb], f16, tag="recip",
                                       name=f"{rp}rc{qb_i}_{h}")
                    with nc.allow_low_precision(reason="softmax recip"):
                        nc.vector.reciprocal(recip[:], ou_sb[64:65, :])
                    norm_jobs.append((qb_i, h, ou_sb, recip))

                if phases >= 3:
                    ph3_jobs += [(qb_i, qb_i * qb // 128 + j)
                                 for j in range(qb // 128)]
            while norm_jobs:
                emit_norm_job(norm_jobs.pop(0))
            while ph3_jobs:
                emit_ph3_job(ph3_jobs.pop(0))

    nc.compile()
    return nc


def get_nc(n_tokens=N, qb=1024, reps=1, phases=3, dve_den=11, dve_num=5):
    key = (n_tokens, qb, reps, phases, dve_den, dve_num)
    if key not in _CACHED_NC:
        _CACHED_NC[key] = _build(n_tokens, qb, reps, phases, dve_den, dve_num)
    return _CACHED_NC[key]


def _chunked(w):
    """[C, d] -> [128, CCH*d]: channel c = a*128 + p lands on partition p,
    chunk a (the on-chip [p, a, d] layout, flattened)."""
    d = w.shape[1]
    return np.ascontiguousarray(
        w.reshape(CCH, 128, d).transpose(1, 0, 2).reshape(128, CCH * d))


def make_in_maps(x, W_qkv, W_proj):
    """Per-core input dicts. Core c = b*4 + g: batch b, heads [3g, 3g+3).
    Host-side prep: fp16 cast + pre-arrangement to on-chip layouts."""
    f16 = np.float16
    x = np.asarray(x, np.float32)
    W_qkv = np.asarray(W_qkv, np.float32)
    W_proj = np.asarray(W_proj, np.float32)
    ntb = N // 512
    # x[b].T [C, N] -> [128, ntb*CCH*512] with index [p, (bt, a, col)]
    xT = [np.ascontiguousarray(
        x[b].T.reshape(CCH, 128, ntb, 512).transpose(1, 2, 0, 3)
        .reshape(128, -1)).astype(f16) for b in range(B)]
    in_maps = []
    for core in range(8):
        b, g = core // 4, core % 4
        h0 = g * NH * D  # column offset of this group's heads
        wk_s = W_qkv[:, C + h0:C + h0 + NH * D].astype(f16)
        wq_s = W_qkv[:, h0:h0 + NH * D].astype(f16)
        wp2 = W_proj[h0 + 128:h0 + 192].astype(f16)  # [64, C]
        in_maps.append({
            "xT_b": xT[b],
            "w_q": _chunked(wq_s),
            "w_k": _chunked(wk_s),
            "w_v": _chunked(
                W_qkv[:, 2 * C + h0:2 * C + h0 + NH * D].astype(f16)),
            "w_kq2": _chunked(
                np.concatenate([wk_s[:, 128:192], wq_s[:, 128:192]], axis=1)),
            "w_p01": np.ascontiguousarray(W_proj[h0:h0 + 128].astype(f16)),
            "w_p2d": np.ascontiguousarray(np.concatenate([wp2, wp2], axis=0)),
        })
    return in_maps


def kernel(x, W_qkv, W_proj, b_proj):
    from concourse.bass_utils import run_bass_kernel_spmd

    nc = get_nc()
    in_maps = make_in_maps(x, W_qkv, W_proj)
    res = run_bass_kernel_spmd(nc, in_maps, core_ids=list(range(8)))
    partials = [np.asarray(res.results[c]["out"], np.float32) for c in range(8)]
    out = np.stack([
        partials[0] + partials[1] + partials[2] + partials[3],
        partials[4] + partials[5] + partials[6] + partials[7],
    ])
    return (out + np.asarray(b_proj, np.float32)).astype(np.float32)

